# revision 11
# baseline (speedup 1.0000x reference)
"""Binarized 3x3 conv (N=32, C=256->256, H=W=56, pad 1) on 8 TRN2 NeuronCores.

Sharding: data-parallel over batch (4 images per core), weights replicated.

Math: binarize exactly via
  xb = (x >= 0) - 0.5            in {+-0.5}  (exact in fp8 e4m3)
  wb = 4*(w >= 0) - 2            in {+-2}    (exact in fp8 e4m3)
so every product is exactly +-1 and fp32 PSUM accumulation is exact
(integer partial sums, |.| <= 2304 << 2^24). sign(0)=+1 is honored.

Conv as matmul: the padded (58x58) binarized image lives flat in SBUF, so for
each kernel tap (kh,kw) the needed input window is a CONTIGUOUS span of the
flat padded grid shifted by (kh-1)*58+(kw-1). Outputs are computed on the
padded grid (464-wide spans = 8 padded rows) and the two garbage columns per
row (conv centered on pad columns) are dropped at drain time.

v2 layout/pipeline changes vs v1:
 - w is host-pre-permuted (pure layout) to [ci_local][cc][tap][two][co] so the
   weight load is ONE contiguous DMA + one DVE binarize pass (v1's strided
   gather needed 32K 36-byte descriptors and stalled the kernel ~70us).
 - x is host-pre-transposed (pure layout) to [ci_local][chunk][img][hw] so the
   input DMA is contiguous per partition, and images load individually so
   image n+1's DMA overlaps image n's matmuls.
 - matmul loop: taps OUTER, psum-bank-set inner (sets of 4/3 row-groups) so
   one LDWEIGHTS feeds 4 matmuls (144 LDW vs 504).
 - two 4-bank PSUM tiles ping-pong so ACT drains overlap the next set's MMs.
"""

import os
os.environ.setdefault("CONCOURSE_SCRUB_NEFF_DEBUG_INFO", "1")

import numpy as np

import concourse.bass as bass
import concourse.mybir as mybir
import concourse.tile as tile
from concourse import bacc, bass_utils

N_CORES = 8
N, CIN, H, W = 32, 256, 56, 56
COUT, KS = 256, 3
NPC = N // N_CORES          # images per core
HP, WP = H + 2, W + 2       # padded spatial (58x58)
GRID = HP * WP              # 3364
LEAD = 64                   # front pad so tap offsets never go negative
CHUNK = 3440                # LEAD + GRID + 12 tail, %16 == 0 (DoubleRow step)
NRG = 7                     # row groups
RPG = H // NRG              # 8 rows per group
FREE = RPG * WP             # 464 <= 512 (one PSUM bank, fp32)
OFREE = RPG * W             # 448 valid output columns
CI_CHUNKS = CIN // 128
CO_CHUNKS = COUT // 128
HW = H * W                  # 3136
SETS = ((0, 1, 2, 3), (4, 5, 6))   # row-group sets -> psum bank sets

F32 = mybir.dt.float32
FP8 = mybir.dt.float8e4
ALU = mybir.AluOpType
AF = mybir.ActivationFunctionType
DR = mybir.MatmulPerfMode.DoubleRow


def _body(tc, x_d, w_d, b_d, o_d, repeats=1, parts='full', loop_trips=None):
    nc = tc.nc

    from contextlib import ExitStack
    ctx = ExitStack()
    with ctx:
        const_pool = ctx.enter_context(tc.tile_pool(name="const", bufs=1))
        xin_pool = ctx.enter_context(tc.tile_pool(name="xin", bufs=1))
        xpad_pool = ctx.enter_context(tc.tile_pool(name="xpad", bufs=1))
        out_pool = ctx.enter_context(tc.tile_pool(name="outs", bufs=1))
        wstage = ctx.enter_context(tc.tile_pool(name="wstage", bufs=1))

        bias_sb = const_pool.tile([128, CO_CHUNKS], F32, tag="bias", name="bias_sb")
        nc.sync.dma_start(bias_sb[:], b_d.rearrange("(c p) -> p c", p=128))

        # ---- persistent slots ----
        NXR, NXP = 2, 3
        xr_slots = [xin_pool.tile([128, CI_CHUNKS * HW], F32, tag=f"xr{s}",
                                  name=f"xr{s}") for s in range(NXR)]
        xp_slots = [xpad_pool.tile([128, CI_CHUNKS * CHUNK], FP8, tag=f"xp{s}",
                                   name=f"xp{s}") for s in range(NXP)]
        ob_slots = [out_pool.tile([128, NRG * OFREE], F32, tag=f"ob{s}",
                                  name=f"ob{s}") for s in range(2)]
        wd8 = const_pool.tile([128, CO_CHUNKS * KS * KS * 256], FP8,
                              tag="wd8", name="wd8")
        wstg = wstage.tile([128, CO_CHUNKS * KS * KS * 256], F32,
                           tag="wstg", name="wstg")

        def dma_x(n_lin):
            # image n_lin (linear over reps); slots rotate mod NXR.
            # src is plain NCHW; the (ci -> partition, chunk) split happens in
            # the DMA access pattern (2 contiguous 12.5KB runs per partition)
            nc.sync.dma_start(
                xr_slots[n_lin % NXR][:]
                .rearrange("c (t s) -> c t s", t=CI_CHUNKS),
                x_d[n_lin % NPC].rearrange("(t k) h w -> k t (h w)",
                                           t=CI_CHUNKS))

        # issue order: image 0 load first, then the (smaller) weight load,
        # split per co-chunk so cc0's matmuls can start before cc1 arrives
        WSZ = KS * KS * 256
        if loop_trips is None:
            dma_x(0)
        for cc in range(CO_CHUNKS):
            nc.sync.dma_start(wstg[:, cc * WSZ:(cc + 1) * WSZ],
                              w_d[:, cc * WSZ:(cc + 1) * WSZ])
        if loop_trips is not None:
            # timing-loop mode: weights binarized once, up front
            for cc in range(CO_CHUNKS):
                nc.vector.tensor_scalar(
                    wd8[:, cc * WSZ:(cc + 1) * WSZ],
                    wstg[:, cc * WSZ:(cc + 1) * WSZ],
                    0.0, 0.5, op0=ALU.is_ge, op1=ALU.subtract)

        # zero the pad borders once; binarize only ever writes the interior
        for s in range(NXP):
            xg = xp_slots[s][:].rearrange("c (g s) -> c g s", s=CHUNK)
            nc.gpsimd.memset(xg[:, :, 0:LEAD], 0.0)
            nc.gpsimd.memset(xg[:, :, LEAD + GRID:CHUNK], 0.0)
            xgrid = xg[:, :, LEAD:LEAD + GRID] \
                .rearrange("c g (h w) -> c g h w", w=WP)
            nc.gpsimd.memset(xgrid[:, :, 0:1, :], 0.0)
            nc.gpsimd.memset(xgrid[:, :, HP - 1:HP, :], 0.0)
            nc.gpsimd.memset(xgrid[:, :, 1:HP - 1, 0:1], 0.0)
            nc.gpsimd.memset(xgrid[:, :, 1:HP - 1, WP - 1:WP], 0.0)

        def binarize_x(n_lin):
            xp = xp_slots[n_lin % NXP]
            nc.vector.tensor_scalar(
                xp[:].rearrange("c (t s) -> c t s", s=CHUNK)
                [:, :, LEAD:LEAD + GRID]
                .rearrange("c t (h w) -> c t h w", w=WP)
                [:, :, 1:H + 1, 1:W + 1],
                xr_slots[n_lin % NXR][:]
                .rearrange("c (t h w) -> c t h w", t=CI_CHUNKS, w=W),
                0.0, 0.5, op0=ALU.is_ge, op1=ALU.subtract)

        with tc.tile_pool(name="cpsum", bufs=1, space="PSUM") as cpsum:
            pp_slots = [cpsum.tile([128, 4 * 512], F32, tag=f"cps{s}",
                                   name=f"cps{s}", bufs=1) for s in range(2)]
            o_d3 = [[o_d[n, cc * 128:(cc + 1) * 128]
                     .rearrange("c h w -> c (h w)")
                     for cc in range(CO_CHUNKS)] for n in range(NPC)]
            set_state = [0]
            do_io = parts in ('full', 'noout', 'input')
            do_mm = parts != 'input'

            def rep_body(rep, in_loop=False):
                set_idx = set_state[0]
                for n in range(NPC):
                    n_lin = rep * NPC + n
                    if in_loop and n == 0:
                        dma_x(0)
                    if do_io or n_lin < NPC:
                        binarize_x(n_lin)
                    if n_lin == 0 and not in_loop:
                        # binarize weights to {+-0.5}; drain rescales by 4
                        for cc in range(CO_CHUNKS):
                            nc.vector.tensor_scalar(
                                wd8[:, cc * WSZ:(cc + 1) * WSZ],
                                wstg[:, cc * WSZ:(cc + 1) * WSZ],
                                0.0, 0.5, op0=ALU.is_ge, op1=ALU.subtract)
                    if in_loop:
                        if n + 1 < NPC:
                            dma_x(n + 1)
                    elif (n_lin + 1 < repeats * NPC
                          and (do_io or n_lin + 1 < NPC)):
                        dma_x(n_lin + 1)
                    if not do_mm:
                        continue
                    xp3 = xp_slots[n_lin % NXP][:] \
                        .rearrange("k (two s) -> k two s", s=CHUNK)

                    for cc in range(CO_CHUNKS):
                        ob = ob_slots[(n * CO_CHUNKS + cc) % 2]
                        for si, rgs in enumerate(SETS):
                            pp = pp_slots[set_idx % 2]
                            set_idx += 1
                            set_state[0] = set_idx
                            if parts == 'tapinner':
                                for j, rg in enumerate(rgs):
                                    for kp in range(KS * KS):
                                        kh, kw = divmod(kp, KS)
                                        lhsT = wd8[:, (cc * KS * KS + kp) * 256:
                                                   (cc * KS * KS + kp + 1) * 256] \
                                            .rearrange("k (two m) -> k two m",
                                                       two=2)
                                        off = (LEAD + WP + rg * FREE
                                               + (kh - 1) * WP + (kw - 1))
                                        nc.tensor.matmul(
                                            pp[:, j * 512:j * 512 + FREE],
                                            lhsT, xp3[:, :, off:off + FREE],
                                            start=(kp == 0),
                                            stop=(kp == KS * KS - 1),
                                            perf_mode=DR)
                            else:
                                for kp in range(KS * KS):
                                    kh, kw = divmod(kp, KS)
                                    lhsT = wd8[:, (cc * KS * KS + kp) * 256:
                                               (cc * KS * KS + kp + 1) * 256] \
                                        .rearrange("k (two m) -> k two m", two=2)
                                    for j, rg in enumerate(rgs):
                                        off = (LEAD + WP + rg * FREE
                                               + (kh - 1) * WP + (kw - 1))
                                        nc.tensor.matmul(
                                            pp[:, j * 512:j * 512 + FREE],
                                            lhsT, xp3[:, :, off:off + FREE],
                                            start=(kp == 0),
                                            stop=(kp == KS * KS - 1),
                                            perf_mode=DR)
                            if parts == 'mmonly' or parts == 'tapinner':
                                continue
                            # drain set: drop the 2 pad columns per row and
                            # rescale the {+-0.25} products back by 4
                            drain_in = pp[:, :len(rgs) * 512].rearrange(
                                "m (g s) -> m g s", s=512)[:, :, :FREE] \
                                .rearrange("m g (r c) -> m g r c", c=WP
                                           )[:, :, :, 1:W + 1]
                            o0, o1 = rgs[0] * OFREE, (rgs[-1] + 1) * OFREE
                            drain_out = ob[:, o0:o1].rearrange(
                                "m (g r c) -> m g r c", g=len(rgs), c=W)
                            nc.scalar.activation(
                                drain_out, drain_in,
                                AF.Identity, bias=bias_sb[:, cc:cc + 1],
                                scale=4.0)
                            # ship each set as soon as it is drained on the
                            # SWDGE (gpsimd) ring so stores don't head-of-line
                            # block the x prefetches on the SP HWDGE ring
                            if parts != 'noout':
                                nc.gpsimd.dma_start(o_d3[n][cc][:, o0:o1],
                                                    ob[:, o0:o1])

            if loop_trips is None:
                for rep in range(repeats):
                    rep_body(rep)
            else:
                with tc.For_i(0, loop_trips, 1):
                    rep_body(0, in_loop=True)


_nc_cache = {}


def _get_nc(repeats=1, parts='full', loop_trips=None):
    key = (repeats, parts, loop_trips)
    if key not in _nc_cache:
        nc = bacc.Bacc("TRN2", debug=False)
        x_d = nc.dram_tensor("x", [NPC, CIN, H, W], F32,
                             kind="ExternalInput").ap()
        w_d = nc.dram_tensor("w", [128, CO_CHUNKS * KS * KS * 256], F32,
                             kind="ExternalInput").ap()
        b_d = nc.dram_tensor("b", [COUT], F32, kind="ExternalInput").ap()
        o_d = nc.dram_tensor("out", [NPC, COUT, H, W], F32,
                             kind="ExternalOutput").ap()
        with tile.TileContext(nc) as tc:
            _body(tc, x_d, w_d, b_d, o_d, repeats=repeats, parts=parts,
                  loop_trips=loop_trips)
        nc.compile()
        _nc_cache[key] = nc
    return _nc_cache[key]


def _prep_w(w):
    # [co][ci][kh][kw] -> [k][cc][tap][two][co_l]; pure layout, no values
    wr = np.asarray(w, dtype=np.float32).reshape(CO_CHUNKS, 128, CI_CHUNKS,
                                                 128, KS * KS)
    wp = wr.transpose(3, 0, 4, 2, 1)          # [k][cc][tap][two][co_l]
    return np.ascontiguousarray(wp).reshape(128, CO_CHUNKS * KS * KS * 256)


def _prep_x(xc):
    # plain per-core batch slice, zero-copy when already contiguous f32
    return np.ascontiguousarray(xc, dtype=np.float32)


def _run(inputs, repeats=1, **kwargs):
    x, w, b = inputs["x"], inputs["w"], inputs["b"]
    assert x.shape == (N, CIN, H, W), x.shape
    nc = _get_nc(repeats)
    wp = _prep_w(w)
    bp = np.ascontiguousarray(b, dtype=np.float32)
    in_maps = [{
        "x": _prep_x(x[i * NPC:(i + 1) * NPC]),
        "w": wp,
        "b": bp,
    } for i in range(N_CORES)]
    res = bass_utils.run_bass_kernel_spmd(
        nc, in_maps, core_ids=list(range(N_CORES)), **kwargs)
    out = np.concatenate([res.results[i]["out"] for i in range(N_CORES)], axis=0)
    return out, res


def kernel(**inputs) -> np.ndarray:
    out, _ = _run(inputs)
    return out


# revision 12
# speedup vs baseline: 1.1312x; 1.1312x over previous
"""Binarized 3x3 conv (N=32, C=256->256, H=W=56, pad 1) on 8 TRN2 NeuronCores.

Sharding: data-parallel over batch (4 images per core), weights replicated.

Math: binarize exactly via
  xb = (x >= 0) - 0.5            in {+-0.5}  (exact in fp8 e4m3)
  wb = 4*(w >= 0) - 2            in {+-2}    (exact in fp8 e4m3)
so every product is exactly +-1 and fp32 PSUM accumulation is exact
(integer partial sums, |.| <= 2304 << 2^24). sign(0)=+1 is honored.

Conv as matmul: the padded (58x58) binarized image lives flat in SBUF, so for
each kernel tap (kh,kw) the needed input window is a CONTIGUOUS span of the
flat padded grid shifted by (kh-1)*58+(kw-1). Outputs are computed on the
padded grid (464-wide spans = 8 padded rows) and the two garbage columns per
row (conv centered on pad columns) are dropped at drain time.

v2 layout/pipeline changes vs v1:
 - w is host-pre-permuted (pure layout) to [ci_local][cc][tap][two][co] so the
   weight load is ONE contiguous DMA + one DVE binarize pass (v1's strided
   gather needed 32K 36-byte descriptors and stalled the kernel ~70us).
 - x is host-pre-transposed (pure layout) to [ci_local][chunk][img][hw] so the
   input DMA is contiguous per partition, and images load individually so
   image n+1's DMA overlaps image n's matmuls.
 - matmul loop: taps OUTER, psum-bank-set inner (sets of 4/3 row-groups) so
   one LDWEIGHTS feeds 4 matmuls (144 LDW vs 504).
 - two 4-bank PSUM tiles ping-pong so ACT drains overlap the next set's MMs.
"""

import os
os.environ.setdefault("CONCOURSE_SCRUB_NEFF_DEBUG_INFO", "1")

import numpy as np

import concourse.bass as bass
import concourse.mybir as mybir
import concourse.tile as tile
from concourse import bacc, bass_utils

N_CORES = 8
N, CIN, H, W = 32, 256, 56, 56
COUT, KS = 256, 3
NPC = N // N_CORES          # images per core
HP, WP = H + 2, W + 2       # padded spatial (58x58)
GRID = HP * WP              # 3364
LEAD = 64                   # front pad so tap offsets never go negative
CHUNK = 3440                # LEAD + GRID + 12 tail, %16 == 0 (DoubleRow step)
NRG = 7                     # row groups
RPG = H // NRG              # 8 rows per group
FREE = RPG * WP             # 464 <= 512 (one PSUM bank, fp32)
OFREE = RPG * W             # 448 valid output columns
CI_CHUNKS = CIN // 128
CO_CHUNKS = COUT // 128
HW = H * W                  # 3136
SETS = ((0, 1, 2, 3), (4, 5, 6))   # row-group sets -> psum bank sets

F32 = mybir.dt.float32
FP8 = mybir.dt.float8e4
ALU = mybir.AluOpType
AF = mybir.ActivationFunctionType
DR = mybir.MatmulPerfMode.DoubleRow


def _body(tc, x_d, w_d, b_d, o_d, repeats=1, parts='full', loop_trips=None):
    nc = tc.nc

    from contextlib import ExitStack
    ctx = ExitStack()
    with ctx:
        const_pool = ctx.enter_context(tc.tile_pool(name="const", bufs=1))
        xin_pool = ctx.enter_context(tc.tile_pool(name="xin", bufs=1))
        xpad_pool = ctx.enter_context(tc.tile_pool(name="xpad", bufs=1))
        out_pool = ctx.enter_context(tc.tile_pool(name="outs", bufs=1))
        wstage = ctx.enter_context(tc.tile_pool(name="wstage", bufs=1))

        bias_sb = const_pool.tile([128, CO_CHUNKS], F32, tag="bias", name="bias_sb")
        nc.sync.dma_start(bias_sb[:], b_d.rearrange("(c p) -> p c", p=128))

        # ---- persistent slots ----
        NXR, NXP = 2, 3
        xr_slots = [xin_pool.tile([128, CI_CHUNKS * HW], F32, tag=f"xr{s}",
                                  name=f"xr{s}") for s in range(NXR)]
        xp_slots = [xpad_pool.tile([128, CI_CHUNKS * CHUNK], FP8, tag=f"xp{s}",
                                   name=f"xp{s}") for s in range(NXP)]
        ob_slots = [out_pool.tile([128, NRG * OFREE], F32, tag=f"ob{s}",
                                  name=f"ob{s}") for s in range(2)]
        wd8 = const_pool.tile([128, CO_CHUNKS * KS * KS * 256], FP8,
                              tag="wd8", name="wd8")
        wstg = wstage.tile([128, CO_CHUNKS * KS * KS * 256], F32,
                           tag="wstg", name="wstg")

        def dma_x(n_lin):
            # image n_lin (linear over reps); slots rotate mod NXR.
            # src is plain NCHW; the (ci -> partition, chunk) split happens in
            # the DMA access pattern (2 contiguous 12.5KB runs per partition)
            nc.sync.dma_start(
                xr_slots[n_lin % NXR][:]
                .rearrange("c (t s) -> c t s", t=CI_CHUNKS),
                x_d[n_lin % NPC].rearrange("(t k) h w -> k t (h w)",
                                           t=CI_CHUNKS))

        # issue order: image 0 load first, then the (smaller) weight load,
        # split per co-chunk so cc0's matmuls can start before cc1 arrives
        WSZ = KS * KS * 256
        if loop_trips is None:
            dma_x(0)
        for cc in range(CO_CHUNKS):
            nc.sync.dma_start(wstg[:, cc * WSZ:(cc + 1) * WSZ],
                              w_d[:, cc * WSZ:(cc + 1) * WSZ])
        if loop_trips is not None:
            # timing-loop mode: weights binarized once, up front
            for cc in range(CO_CHUNKS):
                nc.vector.tensor_scalar(
                    wd8[:, cc * WSZ:(cc + 1) * WSZ],
                    wstg[:, cc * WSZ:(cc + 1) * WSZ],
                    0.0, 0.5, op0=ALU.is_ge, op1=ALU.subtract)

        # zero the pad borders once; binarize only ever writes the interior
        for s in range(NXP):
            xg = xp_slots[s][:].rearrange("c (g s) -> c g s", s=CHUNK)
            nc.gpsimd.memset(xg[:, :, 0:LEAD], 0.0)
            nc.gpsimd.memset(xg[:, :, LEAD + GRID:CHUNK], 0.0)
            xgrid = xg[:, :, LEAD:LEAD + GRID] \
                .rearrange("c g (h w) -> c g h w", w=WP)
            nc.gpsimd.memset(xgrid[:, :, 0:1, :], 0.0)
            nc.gpsimd.memset(xgrid[:, :, HP - 1:HP, :], 0.0)
            nc.gpsimd.memset(xgrid[:, :, 1:HP - 1, 0:1], 0.0)
            nc.gpsimd.memset(xgrid[:, :, 1:HP - 1, WP - 1:WP], 0.0)

        def binarize_x(n_lin):
            xp = xp_slots[n_lin % NXP]
            nc.vector.tensor_scalar(
                xp[:].rearrange("c (t s) -> c t s", s=CHUNK)
                [:, :, LEAD:LEAD + GRID]
                .rearrange("c t (h w) -> c t h w", w=WP)
                [:, :, 1:H + 1, 1:W + 1],
                xr_slots[n_lin % NXR][:]
                .rearrange("c (t h w) -> c t h w", t=CI_CHUNKS, w=W),
                0.0, 0.5, op0=ALU.is_ge, op1=ALU.subtract)

        with tc.tile_pool(name="cpsum", bufs=1, space="PSUM") as cpsum:
            pp_slots = [cpsum.tile([128, 4 * 512], F32, tag=f"cps{s}",
                                   name=f"cps{s}", bufs=1) for s in range(2)]
            o_d3 = [[o_d[n, cc * 128:(cc + 1) * 128]
                     .rearrange("c h w -> c (h w)")
                     for cc in range(CO_CHUNKS)] for n in range(NPC)]
            set_state = [0]
            do_io = parts in ('full', 'noout', 'input')
            do_mm = parts != 'input'

            def rep_body(rep, in_loop=False):
                set_idx = set_state[0]
                for n in range(NPC):
                    n_lin = rep * NPC + n
                    if in_loop and n == 0:
                        dma_x(0)
                    if do_io or n_lin < NPC:
                        binarize_x(n_lin)
                    if n_lin == 0 and not in_loop:
                        # binarize weights to {+-0.5}; drain rescales by 4
                        for cc in range(CO_CHUNKS):
                            nc.vector.tensor_scalar(
                                wd8[:, cc * WSZ:(cc + 1) * WSZ],
                                wstg[:, cc * WSZ:(cc + 1) * WSZ],
                                0.0, 0.5, op0=ALU.is_ge, op1=ALU.subtract)
                    if in_loop:
                        if n + 1 < NPC:
                            dma_x(n + 1)
                    elif (n_lin + 1 < repeats * NPC
                          and (do_io or n_lin + 1 < NPC)):
                        dma_x(n_lin + 1)
                    if not do_mm:
                        continue
                    xp3 = xp_slots[n_lin % NXP][:] \
                        .rearrange("k (two s) -> k two s", s=CHUNK)

                    for cc in range(CO_CHUNKS):
                        ob = ob_slots[(n * CO_CHUNKS + cc) % 2]
                        for si, rgs in enumerate(SETS):
                            pp = pp_slots[set_idx % 2]
                            set_idx += 1
                            set_state[0] = set_idx
                            if parts == 'tapinner':
                                for j, rg in enumerate(rgs):
                                    for kp in range(KS * KS):
                                        kh, kw = divmod(kp, KS)
                                        lhsT = wd8[:, (cc * KS * KS + kp) * 256:
                                                   (cc * KS * KS + kp + 1) * 256] \
                                            .rearrange("k (two m) -> k two m",
                                                       two=2)
                                        off = (LEAD + WP + rg * FREE
                                               + (kh - 1) * WP + (kw - 1))
                                        nc.tensor.matmul(
                                            pp[:, j * 512:j * 512 + FREE],
                                            lhsT, xp3[:, :, off:off + FREE],
                                            start=(kp == 0),
                                            stop=(kp == KS * KS - 1),
                                            perf_mode=DR)
                            else:
                                for kp in range(KS * KS):
                                    kh, kw = divmod(kp, KS)
                                    lhsT = wd8[:, (cc * KS * KS + kp) * 256:
                                               (cc * KS * KS + kp + 1) * 256] \
                                        .rearrange("k (two m) -> k two m", two=2)
                                    for j, rg in enumerate(rgs):
                                        off = (LEAD + WP + rg * FREE
                                               + (kh - 1) * WP + (kw - 1))
                                        nc.tensor.matmul(
                                            pp[:, j * 512:j * 512 + FREE],
                                            lhsT, xp3[:, :, off:off + FREE],
                                            start=(kp == 0),
                                            stop=(kp == KS * KS - 1),
                                            perf_mode=DR)
                            if parts == 'mmonly' or parts == 'tapinner':
                                continue
                            # drain set: drop the 2 pad columns per row and
                            # rescale the {+-0.25} products back by 4
                            drain_in = pp[:, :len(rgs) * 512].rearrange(
                                "m (g s) -> m g s", s=512)[:, :, :FREE] \
                                .rearrange("m g (r c) -> m g r c", c=WP
                                           )[:, :, :, 1:W + 1]
                            o0, o1 = rgs[0] * OFREE, (rgs[-1] + 1) * OFREE
                            drain_out = ob[:, o0:o1].rearrange(
                                "m (g r c) -> m g r c", g=len(rgs), c=W)
                            nc.scalar.activation(
                                drain_out, drain_in,
                                AF.Identity, bias=bias_sb[:, cc:cc + 1],
                                scale=4.0)
                            # ship each set as soon as it is drained on the
                            # SWDGE (gpsimd) ring so stores don't head-of-line
                            # block the x prefetches on the SP HWDGE ring
                            if parts != 'noout':
                                nc.gpsimd.dma_start(o_d3[n][cc][:, o0:o1],
                                                    ob[:, o0:o1])

            if loop_trips is None:
                for rep in range(repeats):
                    rep_body(rep)
            elif parts == 'stag':
                with tc.For_i(0, loop_trips, 1, staggered_reset=True,
                              hint_engines=(mybir.EngineType.PE,)):
                    rep_body(0, in_loop=True)
            else:
                with tc.For_i(0, loop_trips, 1):
                    rep_body(0, in_loop=True)


_nc_cache = {}


def _get_nc(repeats=1, parts='full', loop_trips=None):
    key = (repeats, parts, loop_trips)
    if key not in _nc_cache:
        nc = bacc.Bacc("TRN2", debug=False)
        x_d = nc.dram_tensor("x", [NPC, CIN, H, W], F32,
                             kind="ExternalInput").ap()
        w_d = nc.dram_tensor("w", [128, CO_CHUNKS * KS * KS * 256], F32,
                             kind="ExternalInput").ap()
        b_d = nc.dram_tensor("b", [COUT], F32, kind="ExternalInput").ap()
        o_d = nc.dram_tensor("out", [NPC, COUT, H, W], F32,
                             kind="ExternalOutput").ap()
        with tile.TileContext(nc) as tc:
            _body(tc, x_d, w_d, b_d, o_d, repeats=repeats, parts=parts,
                  loop_trips=loop_trips)
        nc.compile()
        _nc_cache[key] = nc
    return _nc_cache[key]


def _prep_w(w):
    # [co][ci][kh][kw] -> [k][cc][tap][two][co_l]; pure layout, no values
    wr = np.asarray(w, dtype=np.float32).reshape(CO_CHUNKS, 128, CI_CHUNKS,
                                                 128, KS * KS)
    wp = wr.transpose(3, 0, 4, 2, 1)          # [k][cc][tap][two][co_l]
    return np.ascontiguousarray(wp).reshape(128, CO_CHUNKS * KS * KS * 256)


def _prep_x(xc):
    # plain per-core batch slice, zero-copy when already contiguous f32
    return np.ascontiguousarray(xc, dtype=np.float32)


def _run(inputs, repeats=1, **kwargs):
    x, w, b = inputs["x"], inputs["w"], inputs["b"]
    assert x.shape == (N, CIN, H, W), x.shape
    nc = _get_nc(repeats)
    wp = _prep_w(w)
    bp = np.ascontiguousarray(b, dtype=np.float32)
    in_maps = [{
        "x": _prep_x(x[i * NPC:(i + 1) * NPC]),
        "w": wp,
        "b": bp,
    } for i in range(N_CORES)]
    res = bass_utils.run_bass_kernel_spmd(
        nc, in_maps, core_ids=list(range(N_CORES)), **kwargs)
    out = np.concatenate([res.results[i]["out"] for i in range(N_CORES)], axis=0)
    return out, res


def kernel(**inputs) -> np.ndarray:
    out, _ = _run(inputs)
    return out
